# revision 2
# baseline (speedup 1.0000x reference)
"""Trainium2 Bass kernel for nn_AutoRegressiveDistribution (MADE sampling).

Self-contained: hardcodes shapes/sharding. Shards batch B across 8 cores,
runs the D-step autoregressive sampling loop fully on-device per core.

Per-core structure (rows = S*BS = 512, processed as TWO independent
half-chains of an s-pair each so the serial per-step dependency chains
overlap across engines). v2 critical-path redesign:
  - z history is kept bf16 in a 32-blocked layout z2 (b, h, s, beta) and
    bridged to the matmul-consumable layout zq (32g+beta, h, s, a) by ONE
    DVE StreamTranspose of the current 32-degree window per step (SBUF->
    SBUF, ~130ns) instead of PE transposes + a PSUM->SBUF copy (~760ns).
    The hist matmul contracts over (g,beta) partitions per batch-group g:
    permuting the contraction dim identically in lhsT (W1Tb) and rhs (zq)
    leaves the sum invariant, so 4-8 small bf16 MMs replace one big one
    (matmul cost is out-free-size bound, so total PE time is unchanged).
  - The z-update z_i = mu_i + sc_i*eps_i runs on the GPSIMD/Pool engine as
    two fused scalar_tensor_tensor ops (one per s): Pool has no SBUF/PSUM
    access-latency penalty, so reading mu from PSUM is cheap there.
  - softplus = Exp then Ln(1+x) on the Act engine, entirely in PSUM
    (scratch col 64 of the sc tile), avoiding the 222-cycle SBUF access.
  - Only the 2 output columns needed for the z-update ({i, D+i}) are
    accumulated on the critical path (narrow contrib MM); the remaining
    columns (> i) are added by a deferred wide MM off the critical path.
  - ctx_h = Wc @ ctx + b1 is precomputed once (f32r identity-shift seed
    MMs); bout is seeded into OUT once via ones-outer-product matmuls.
"""

import numpy as np
from contextlib import ExitStack

import concourse.bass as bass
import concourse.tile as tile
from concourse import bacc, mybir
from concourse.bass_utils import run_bass_kernel_spmd

D, H, CTX, B, S = 64, 1024, 256, 1024, 4
NCORES = 8
BS = B // NCORES          # 128 batch rows per core
R = S * BS                # 512 rows per core
RH = R // 2               # rows per half-chain (s-pair)
FP32 = mybir.dt.float32
BF16 = mybir.dt.bfloat16
F32R = mybir.dt.float32r

HP = 2048  # padded hidden units: degree block i at [32*(i-1), 32*(i-1)+cnt[i])


def _made_struct():
    mh = (np.arange(H) % (D - 1)) + 1            # degrees 1..63
    perm = np.argsort(mh, kind="stable")
    mh_s = mh[perm]
    cnt = np.bincount(mh_s, minlength=D)          # cnt[d] = #units of degree d
    off = np.concatenate([[0], np.cumsum(cnt)[:-1]]).astype(np.int64)
    return mh, perm, mh_s, cnt, off


def _prep_weights(W1, b1, Wc, Wout):
    """Mask + permute + 32-pad weights host-side (cheap, O(weight size))."""
    mh, perm, mh_s, cnt, off = _made_struct()
    m0 = np.arange(1, D + 1)
    M1 = (mh[:, None] >= m0[None, :]).astype(np.float32)          # (H, D)
    mout = np.concatenate([m0, m0])                                # (2D,)
    Mout = (mout[:, None] > mh[None, :]).astype(np.float32)        # (2D, H)
    W1m = (W1 * M1)[perm]                   # (H, D) permuted rows
    Woutm = (Wout * Mout)[:, perm]          # (2D, H) permuted cols
    src = np.arange(H)
    pdst = 32 * (mh_s - 1) + (src - off[mh_s])   # padded slot of sorted unit
    import ml_dtypes
    bf = ml_dtypes.bfloat16
    W1T = np.zeros((D, HP), np.float32)
    W1T[:, pdst] = W1m.T
    # blocked lhsT for the hist MMs: W1Tb[beta, h, u] = W1T[32h+beta, u]
    W1Tb = np.ascontiguousarray(
        W1T.reshape(2, 32, HP).transpose(1, 0, 2)).astype(bf)
    WcT = np.zeros((CTX, HP), np.float32)
    WcT[:, pdst] = Wc[perm].T
    WcT = WcT.astype(bf)
    b1p = np.zeros((HP, 1), np.float32)
    b1p[pdst, 0] = b1[perm]
    WoutB = np.zeros((32, D - 1, 2 * D), np.float32)  # (slot, block, outcol)
    WoutB[pdst % 32, (mh_s - 1)] = Woutm[:, :].T[src]
    return W1Tb, WoutB.astype(bf), WcT, b1p


_PROGRAM_CACHE = None


def _pin_act_table():
    """Make Exp/Ln/Relu resolvable only via natural_log_exp_and_others so
    the act-table chooser doesn't thrash between the exp and ln tables
    (each LoadActFuncSet costs ~1.3us). Table positions are preserved so
    act_func_set_id stays consistent with act_info.json."""
    import concourse.bacc as bacc_mod
    from concourse import hw_specs
    orig = hw_specs.get_activation_tables
    AF = mybir.ActivationFunctionType
    pin = {AF.Exp, AF.Ln, AF.Relu}

    def filtered(arch):
        out = {}
        for name, fns in orig(arch).items():
            if name == "natural_log_exp_and_others":
                out[name] = set(fns)
            else:
                out[name] = set(fns) - pin
        return out

    bacc_mod.get_activation_tables = filtered


def _build_program():
    """Build + compile the SPMD Bass program (input-independent, cached)."""
    global _PROGRAM_CACHE
    if _PROGRAM_CACHE is not None:
        return _PROGRAM_CACHE
    _pin_act_table()
    _, _, mh_s, cnt, off = _made_struct()

    nc = bacc.Bacc("TRN2", target_bir_lowering=False, debug=False,
                   num_devices=NCORES)

    ctx_d = nc.dram_tensor("ctx", (BS, CTX), FP32, kind="ExternalInput")
    eps_d = nc.dram_tensor("eps", (S, BS, D), FP32, kind="ExternalInput")
    w1tb_d = nc.dram_tensor("w1tb", (32, 2, HP), BF16, kind="ExternalInput")
    woutt_d = nc.dram_tensor("woutt", (32, D - 1, 2 * D), BF16,
                             kind="ExternalInput")
    wct_d = nc.dram_tensor("wct", (CTX, HP), BF16, kind="ExternalInput")
    b1_d = nc.dram_tensor("b1", (HP, 1), FP32, kind="ExternalInput")
    boutb_d = nc.dram_tensor("boutb", (128, 2 * D), FP32, kind="ExternalInput")
    ident_d = nc.dram_tensor("ident", (128, 128), F32R, kind="ExternalInput")
    z_d = nc.dram_tensor("z_out", (S, BS, D), FP32, kind="ExternalOutput")
    mu_d = nc.dram_tensor("mu_out", (S, BS, D), FP32, kind="ExternalOutput")
    sc_d = nc.dram_tensor("sc_out", (S, BS, D), FP32, kind="ExternalOutput")

    AF = mybir.ActivationFunctionType
    OP = mybir.AluOpType

    with tile.TileContext(nc) as tc, ExitStack() as ctx:
        singles = ctx.enter_context(tc.tile_pool(name="singles", bufs=1))
        ablk_pool = ctx.enter_context(tc.tile_pool(name="ablk", bufs=3))
        scratch = ctx.enter_context(tc.tile_pool(name="scratch", bufs=3))
        psPre = ctx.enter_context(tc.tile_pool(name="psPre", bufs=2,
                                               space="PSUM"))
        psA = ctx.enter_context(tc.tile_pool(name="psA", bufs=2, space="PSUM"))
        psOut = ctx.enter_context(tc.tile_pool(name="psOut", bufs=1,
                                               space="PSUM"))
        psSc = ctx.enter_context(tc.tile_pool(name="psSc", bufs=1,
                                              space="PSUM"))

        # ---- load inputs/constants into SBUF (spread across the SP,
        # Act and GPSIMD DGE queues so descriptor generation overlaps) ----
        wct_sb = singles.tile([128, 2, HP], BF16)
        for q in range(4):
            nc.sync.dma_start(
                wct_sb[:, :, 512 * q:512 * q + 512],
                wct_d.ap()[:, 512 * q:512 * q + 512].rearrange(
                    "(k p) h -> p k h", p=128))
        ctx_sb = singles.tile([BS, CTX], FP32)
        nc.scalar.dma_start(ctx_sb[:], ctx_d.ap())
        ident_sb = singles.tile([128, 128], F32R)
        nc.scalar.dma_start(ident_sb[:], ident_d.ap())
        b1_sb = singles.tile([128, HP // 128], FP32)
        nc.scalar.dma_start(b1_sb[:],
                            b1_d.ap().rearrange("(c p) one -> p (c one)",
                                                p=128))
        boutb_sb = singles.tile([128, 2 * D], FP32)
        nc.scalar.dma_start(boutb_sb[:], boutb_d.ap())
        w1tb_sb = singles.tile([32, 2, HP], BF16)
        nc.sync.dma_start(w1tb_sb[:], w1tb_d.ap())
        woutt_sb = singles.tile([32, D - 1, 2 * D], BF16)
        nc.sync.dma_start(woutt_sb[:], woutt_d.ap())
        eps2 = [singles.tile([BS, 2, D], FP32, tag=f"eps{h}", name=f"eps{h}")
                for h in (0, 1)]
        for h in (0, 1):
            nc.gpsimd.dma_start(
                eps2[h][:],
                eps_d.ap()[2 * h:2 * h + 2].rearrange("s b d -> b s d"))
        boutbb_sb = singles.tile([1, 2 * D], BF16)
        nc.vector.tensor_copy(boutbb_sb[:], boutb_sb[0:1, :])
        identb_sb = singles.tile([128, 128], BF16)
        nc.vector.tensor_copy(identb_sb[:], ident_sb[:])
        ones_sb = singles.tile([1, 128], BF16)
        nc.vector.memset(ones_sb[:], 1.0)

        # ---- ctx in bf16 (rounded copy) for the transpose ----
        ctxr_sb = singles.tile([BS, CTX], BF16)
        nc.vector.tensor_copy(ctxr_sb[:], ctx_sb[:])

        # ---- ctxT: (BS, CTX) -> (CTX, BS) in 2 chunks ----
        ctxT_sb = singles.tile([128, 2, BS], BF16)
        for k in range(2):
            ps = psPre.tile([128, BS], BF16, tag="apspre")
            nc.tensor.transpose(ps[:], ctxr_sb[:, k * 128:(k + 1) * 128],
                                identb_sb[:])
            nc.vector.tensor_copy(ctxT_sb[:, k, :], ps[:])

        # ---- A_base = WcT.T @ ctxT + b1 : (HP, BS) in 16 unit-chunks ----
        NCH = HP // 128
        a_base = singles.tile([128, NCH, BS], F32R)
        for hc in range(NCH):
            ps = psPre.tile([128, BS], FP32, tag="apspre")
            for k in range(2):
                nc.tensor.matmul(
                    ps[:],
                    wct_sb[:, k, hc * 128:(hc + 1) * 128],
                    ctxT_sb[:, k, :],
                    start=(k == 0), stop=(k == 1))
            nc.vector.tensor_scalar_add(a_base[:, hc, :], ps[:],
                                        b1_sb[:, hc:hc + 1])

        # ---- per-half state ----
        # z2: (b, hwin, s, beta) bf16; zq: (32g+beta, hwin, s, a) bf16
        z2 = [singles.tile([BS, 2, 64], BF16, tag=f"z{h}", name=f"z{h}")
              for h in (0, 1)]
        zq2 = [[singles.tile([32, 4, 2, 32], BF16, tag=f"zq{h}w{w}",
                             name=f"zq{h}w{w}") for w in (0, 1)]
               for h in (0, 1)]
        mu2 = [singles.tile([BS, 2, D], FP32, tag=f"mu{h}", name=f"mu{h}")
               for h in (0, 1)]
        sc2 = [singles.tile([BS, 2, D], FP32, tag=f"sc{h}", name=f"sc{h}")
               for h in (0, 1)]
        zf2 = [singles.tile([BS, 2, D], FP32, tag=f"zf{h}", name=f"zf{h}")
               for h in (0, 1)]
        outr = psOut.tile([128, 4, 128], FP32, tag="outr", name="outr")
        # sc in PSUM: cols 0..63 = scale_i, col 64 = exp scratch
        scps = psSc.tile([128, 4, 65], FP32, tag="scps", name="scps")

        for h in (0, 1):
            nc.vector.memset(z2[h][:], 0.0)
            nc.vector.memset(zq2[h][0][:], 0.0)
            nc.vector.memset(zq2[h][1][:], 0.0)

        def bridge(i, h):
            """StreamTranspose the 32-degree window containing col i from
            z-batch layout into matmul rhs layout: zq[beta, w, g, s, a] =
            z[32g+a, s, 32w+beta] (4 per-batch-group block transposes,
            SBUF->SBUF, all landing at partitions 0..31)."""
            w = i // 32
            for g in range(4):
                nc.vector.transpose(zq2[h][w][0:32, g, :, :],
                                    z2[h][32 * g:32 * g + 32, w, :])

        def zupdate(i, h, mu_ap):
            """z_i = mu_i + sc_i*eps_i as Act ops (one per s)."""
            w, beta = i // 32, i % 32
            for s in (0, 1):
                nc.scalar.activation(
                    out=z2[h][:, w, 32 * s + beta:32 * s + beta + 1],
                    in_=scps[:, 2 * h + s, i:i + 1],
                    func=AF.Identity,
                    bias=mu_ap(s),
                    scale=eps2[h][:, s, i:i + 1])

        # ---- step 0 (bias-only): mu0 = bout[0], sc0 = softplus(bout[D]) ----
        for h in (0, 1):
            nc.scalar.activation(out=scps[:, 2 * h:2 * h + 2, 64],
                                 in_=boutb_sb[:, D:D + 1].to_broadcast((BS, 2)),
                                 func=AF.Exp, bias=0.0, scale=1.0)
            nc.scalar.activation(out=scps[:, 2 * h:2 * h + 2, 0],
                                 in_=scps[:, 2 * h:2 * h + 2, 64],
                                 func=AF.Ln, bias=1.0, scale=1.0)
            zupdate(0, h, lambda s: boutb_sb[:, 0:1])
            bridge(0, h)

        # ---- seed OUT with bout once (ones x bout row); single psum
        # accumulation group for the whole shared bank ----
        for hs in range(4):
            nc.tensor.matmul(outr[:, hs, :], ones_sb[:], boutbb_sb[:],
                             start=(hs == 0), stop=False,
                             skip_group_check=True)

        # ---- steps 1..63, two interleaved half-chains ----
        for i in range(1, D):
            nn = int(cnt[i])
            pp = 32 * (i - 1)              # padded unit offset of block i
            c, pl = pp // 128, pp % 128
            q = min(pl, 64)                # 32-aligned base (96 -> 64)
            kk = pl - q + nn
            wfull = i // 32                # complete 32-windows in history
            m = i % 32                     # rows of the partial window
            for h in (0, 1):
                # fresh block pre-activation = ctx_base (shifted-identity MM)
                # + sum over history windows of W1Tb.T @ zq
                aps_t = psA.tile([32, 256], FP32, tag=f"aps{h}")
                sl = a_base[q:q + kk, c, :]
                rhs = bass.AP(sl.tensor, sl.offset,
                              [sl.ap[0], [32, 4], [0, 2], [1, 32]])
                nc.tensor.matmul(aps_t[0:nn, :],
                                 ident_sb[q:q + kk, pl:pl + nn],
                                 rhs, start=True, stop=False,
                                 skip_group_check=True)
                nmm = (1 if wfull else 0) + (1 if m else 0)
                j = 0
                for w in range(wfull):     # complete windows
                    j += 1
                    nc.tensor.matmul(
                        aps_t[0:nn, :],
                        w1tb_sb[0:32, w, pp:pp + nn],
                        zq2[h][w][0:32, :, :, :],
                        start=False, stop=(j == nmm),
                        skip_group_check=True)
                if m:                      # partial window
                    j += 1
                    nc.tensor.matmul(
                        aps_t[0:nn, :],
                        w1tb_sb[0:m, wfull, pp:pp + nn],
                        zq2[h][wfull][0:m, :, :, :],
                        start=False, stop=(j == nmm),
                        skip_group_check=True)

                # relu -> bf16 on Act, reordering (g,s,a) -> (s,g,a) so
                # the contrib lhsT per s is a contiguous 128-col slice
                ab = ablk_pool.tile([32, 256], BF16, tag=f"ablk{h}")
                asl = aps_t[0:nn, 0:1]
                ain0 = bass.AP(asl.tensor, asl.offset,
                               [asl.ap[0], [64, 4], [1, 32]])
                ain1 = bass.AP(asl.tensor, asl.offset + 32,
                               [asl.ap[0], [64, 4], [1, 32]])
                nc.scalar.activation(out=ab[0:nn, 128:256], in_=ain1,
                                     func=AF.Relu, bias=0.0, scale=1.0)
                nc.vector.tensor_scalar_max(ab[0:nn, 0:128], ain0, 0.0)

                # per-s tail, entirely on Act in (128,1) columns (zero
                # marginal engine time): narrow contrib -> mu copy to SBUF
                # -> softplus -> z = Identity(sc*eps + mu)
                wsl = woutt_sb[0:nn, i - 1, :]
                w_, beta = i // 32, i % 32
                musb = scratch.tile([BS, 2], FP32, tag=f"mu{h}s")
                for s in (1, 0):
                    lhs_sl = ab[0:nn, 128 * s:128 * s + 128]
                    wap_n = bass.AP(wsl.tensor, wsl.offset + i,
                                    [wsl.ap[0], [D, 2], [1, 1]])
                    osl = outr[:, 2 * h + s, :]
                    oap_n = bass.AP(osl.tensor, osl.offset + i,
                                    [osl.ap[0], [D, 2], [1, 1]])
                    nc.tensor.matmul(oap_n, lhs_sl, wap_n,
                                     start=False,
                                     stop=(i == D - 1 and s == 0 and h == 1),
                                     skip_group_check=True)
                    nc.scalar.activation(out=musb[:, s:s + 1],
                                         in_=outr[:, 2 * h + s, i:i + 1],
                                         func=AF.Copy, bias=0.0, scale=1.0)
                    nc.scalar.activation(out=scps[:, 2 * h + s, 64:65],
                                         in_=outr[:, 2 * h + s,
                                                  D + i:D + i + 1],
                                         func=AF.Exp, bias=0.0, scale=1.0)
                    nc.scalar.activation(out=scps[:, 2 * h + s, i:i + 1],
                                         in_=scps[:, 2 * h + s, 64:65],
                                         func=AF.Ln, bias=1.0, scale=1.0)
                    nc.scalar.activation(
                        out=z2[h][:, w_, 32 * s + beta:32 * s + beta + 1],
                        in_=scps[:, 2 * h + s, i:i + 1],
                        func=AF.Identity,
                        bias=musb[:, s:s + 1],
                        scale=eps2[h][:, s, i:i + 1])
                if i < D - 1:
                    bridge(i, h)

                # deferred wide contribution: cols {i+1..D-1, D+i+1..2D-1}
                if i < D - 1:
                    for s in (0, 1):
                        lhs_sl = ab[0:nn, 128 * s:128 * s + 128]
                        wap_w = bass.AP(wsl.tensor, wsl.offset + i + 1,
                                        [wsl.ap[0], [D, 2], [1, D - 1 - i]])
                        osl = outr[:, 2 * h + s, :]
                        oap_w = bass.AP(osl.tensor, osl.offset + i + 1,
                                        [osl.ap[0], [D, 2], [1, D - 1 - i]])
                        nc.tensor.matmul(oap_w, lhs_sl, wap_w,
                                         start=False, stop=False,
                                         skip_group_check=True)

        # ---- mu/sc extraction (batched) + outputs ----
        for h in (0, 1):
            nc.vector.tensor_copy(mu2[h][:, :, 0],
                                  boutb_sb[:, 0:1].to_broadcast((BS, 2)))
            nc.vector.tensor_copy(mu2[h][:, :, 1:D], outr[:, 2 * h:2 * h + 2, 1:D])
            nc.vector.tensor_copy(sc2[h][:], scps[:, 2 * h:2 * h + 2, 0:D])
            # zf[b, s, d] = z2[b, d//32, 32*s + d%32] (bf16->f32 gather)
            zsl = z2[h][:, 0, 0:1]
            zin = bass.AP(zsl.tensor, zsl.offset,
                          [zsl.ap[0], [32, 2], [64, 2], [1, 32]])
            fsl = zf2[h][:, 0, 0:1]
            fout = bass.AP(fsl.tensor, fsl.offset,
                           [fsl.ap[0], [64, 2], [32, 2], [1, 32]])
            nc.vector.tensor_copy(fout, zin)
            nc.sync.dma_start(
                z_d.ap()[2 * h:2 * h + 2].rearrange("s b d -> b s d"),
                zf2[h][:])
            nc.gpsimd.dma_start(
                mu_d.ap()[2 * h:2 * h + 2].rearrange("s b d -> b s d"),
                mu2[h][:])
            nc.scalar.dma_start(
                sc_d.ap()[2 * h:2 * h + 2].rearrange("s b d -> b s d"),
                sc2[h][:])

    nc.compile()
    _PROGRAM_CACHE = nc
    return nc


def _in_maps(context, eps, W1, b1, Wc, Wout, bout):
    W1Tb, WoutB, WcT, b1p = _prep_weights(W1, b1, Wc, Wout)
    ident = np.eye(128, dtype=np.float32)
    boutb = np.ascontiguousarray(np.tile(bout.reshape(1, -1), (128, 1)))
    maps = []
    for c in range(NCORES):
        maps.append({
            "ctx": np.ascontiguousarray(context[c * BS:(c + 1) * BS]),
            "eps": np.ascontiguousarray(eps[:, c * BS:(c + 1) * BS]),
            "w1tb": W1Tb, "woutt": WoutB, "wct": WcT, "b1": b1p,
            "boutb": boutb, "ident": ident,
        })
    return maps


def run(context, eps, W1, b1, Wc, Wout, bout, trace=False):
    context = np.asarray(context, np.float32)
    eps = np.asarray(eps, np.float32)
    W1 = np.asarray(W1, np.float32)
    b1 = np.asarray(b1, np.float32)
    Wc = np.asarray(Wc, np.float32)
    Wout = np.asarray(Wout, np.float32)
    bout = np.asarray(bout, np.float32)
    nc = _build_program()
    maps = _in_maps(context, eps, W1, b1, Wc, Wout, bout)
    res = run_bass_kernel_spmd(nc, maps, core_ids=list(range(NCORES)),
                               trace=trace)
    z = np.empty((S, B, D), np.float32)
    mu = np.empty((S, B, D), np.float32)
    sc = np.empty((S, B, D), np.float32)
    for c in range(NCORES):
        z[:, c * BS:(c + 1) * BS] = res.results[c]["z_out"]
        mu[:, c * BS:(c + 1) * BS] = res.results[c]["mu_out"]
        sc[:, c * BS:(c + 1) * BS] = res.results[c]["sc_out"]
    return (z, mu, sc), res


def kernel(context, eps, W1, b1, Wc, Wout, bout):
    (z, mu, sc), _ = run(context, eps, W1, b1, Wc, Wout, bout)
    return z, mu, sc


# revision 3
# speedup vs baseline: 1.4107x; 1.4107x over previous
"""Trainium2 Bass kernel for nn_AutoRegressiveDistribution (MADE sampling).

Self-contained: hardcodes shapes/sharding. Shards batch B across 8 cores,
runs the D-step autoregressive sampling loop fully on-device per core.

Per-core structure (rows = S*BS = 512, processed as TWO independent
half-chains of an s-pair each so the serial per-step dependency chains
overlap across engines). v2 critical-path redesign:
  - z history is kept bf16 in a 32-blocked layout z2 (b, h, s, beta) and
    bridged to the matmul-consumable layout zq (32g+beta, h, s, a) by ONE
    DVE StreamTranspose of the current 32-degree window per step (SBUF->
    SBUF, ~130ns) instead of PE transposes + a PSUM->SBUF copy (~760ns).
    The hist matmul contracts over (g,beta) partitions per batch-group g:
    permuting the contraction dim identically in lhsT (W1Tb) and rhs (zq)
    leaves the sum invariant, so 4-8 small bf16 MMs replace one big one
    (matmul cost is out-free-size bound, so total PE time is unchanged).
  - The z-update z_i = mu_i + sc_i*eps_i runs on the GPSIMD/Pool engine as
    two fused scalar_tensor_tensor ops (one per s): Pool has no SBUF/PSUM
    access-latency penalty, so reading mu from PSUM is cheap there.
  - softplus = Exp then Ln(1+x) on the Act engine, entirely in PSUM
    (scratch col 64 of the sc tile), avoiding the 222-cycle SBUF access.
  - Only the 2 output columns needed for the z-update ({i, D+i}) are
    accumulated on the critical path (narrow contrib MM); the remaining
    columns (> i) are added by a deferred wide MM off the critical path.
  - ctx_h = Wc @ ctx + b1 is precomputed once (f32r identity-shift seed
    MMs); bout is seeded into OUT once via ones-outer-product matmuls.
"""

import numpy as np
from contextlib import ExitStack

import concourse.bass as bass
import concourse.tile as tile
from concourse import bacc, mybir
from concourse.bass_utils import run_bass_kernel_spmd

D, H, CTX, B, S = 64, 1024, 256, 1024, 4
NCORES = 8
BS = B // NCORES          # 128 batch rows per core
R = S * BS                # 512 rows per core
RH = R // 2               # rows per half-chain (s-pair)
FP32 = mybir.dt.float32
BF16 = mybir.dt.bfloat16
F32R = mybir.dt.float32r

HP = 2048  # padded hidden units: degree block i at [32*(i-1), 32*(i-1)+cnt[i])


def _made_struct():
    mh = (np.arange(H) % (D - 1)) + 1            # degrees 1..63
    perm = np.argsort(mh, kind="stable")
    mh_s = mh[perm]
    cnt = np.bincount(mh_s, minlength=D)          # cnt[d] = #units of degree d
    off = np.concatenate([[0], np.cumsum(cnt)[:-1]]).astype(np.int64)
    return mh, perm, mh_s, cnt, off


def _prep_weights(W1, b1, Wc, Wout):
    """Mask + permute + 32-pad weights host-side (cheap, O(weight size))."""
    mh, perm, mh_s, cnt, off = _made_struct()
    m0 = np.arange(1, D + 1)
    M1 = (mh[:, None] >= m0[None, :]).astype(np.float32)          # (H, D)
    mout = np.concatenate([m0, m0])                                # (2D,)
    Mout = (mout[:, None] > mh[None, :]).astype(np.float32)        # (2D, H)
    W1m = (W1 * M1)[perm]                   # (H, D) permuted rows
    Woutm = (Wout * Mout)[:, perm]          # (2D, H) permuted cols
    src = np.arange(H)
    pdst = 32 * (mh_s - 1) + (src - off[mh_s])   # padded slot of sorted unit
    import ml_dtypes
    bf = ml_dtypes.bfloat16
    W1T = np.zeros((D, HP), np.float32)
    W1T[:, pdst] = W1m.T
    # blocked lhsT for the hist MMs: W1Tb[beta, h, u] = W1T[32h+beta, u]
    W1Tb = np.ascontiguousarray(
        W1T.reshape(2, 32, HP).transpose(1, 0, 2)).astype(bf)
    WcT = np.zeros((CTX, HP), np.float32)
    WcT[:, pdst] = Wc[perm].T
    WcT = WcT.astype(bf)
    b1p = np.zeros((HP, 1), np.float32)
    b1p[pdst, 0] = b1[perm]
    WoutB = np.zeros((32, D - 1, 2 * D), np.float32)  # (slot, block, outcol)
    WoutB[pdst % 32, (mh_s - 1)] = Woutm[:, :].T[src]
    return W1Tb, WoutB.astype(bf), WcT, b1p


_PROGRAM_CACHE = None


def _pin_act_table():
    """Make Exp/Ln/Relu resolvable only via natural_log_exp_and_others so
    the act-table chooser doesn't thrash between the exp and ln tables
    (each LoadActFuncSet costs ~1.3us). Table positions are preserved so
    act_func_set_id stays consistent with act_info.json."""
    import concourse.bacc as bacc_mod
    from concourse import hw_specs
    orig = hw_specs.get_activation_tables
    AF = mybir.ActivationFunctionType
    pin = {AF.Exp, AF.Ln, AF.Relu}

    def filtered(arch):
        out = {}
        for name, fns in orig(arch).items():
            if name == "natural_log_exp_and_others":
                out[name] = set(fns)
            else:
                out[name] = set(fns) - pin
        return out

    bacc_mod.get_activation_tables = filtered


def _build_program():
    """Build + compile the SPMD Bass program (input-independent, cached)."""
    global _PROGRAM_CACHE
    if _PROGRAM_CACHE is not None:
        return _PROGRAM_CACHE
    _pin_act_table()
    _, _, mh_s, cnt, off = _made_struct()

    nc = bacc.Bacc("TRN2", target_bir_lowering=False, debug=False,
                   num_devices=NCORES)

    ctx_d = nc.dram_tensor("ctx", (BS, CTX), FP32, kind="ExternalInput")
    eps_d = nc.dram_tensor("eps", (S, BS, D), FP32, kind="ExternalInput")
    w1tb_d = nc.dram_tensor("w1tb", (32, 2, HP), BF16, kind="ExternalInput")
    woutt_d = nc.dram_tensor("woutt", (32, D - 1, 2 * D), BF16,
                             kind="ExternalInput")
    wct_d = nc.dram_tensor("wct", (CTX, HP), BF16, kind="ExternalInput")
    b1_d = nc.dram_tensor("b1", (HP, 1), FP32, kind="ExternalInput")
    boutb_d = nc.dram_tensor("boutb", (128, 2 * D), FP32, kind="ExternalInput")
    ident_d = nc.dram_tensor("ident", (128, 128), F32R, kind="ExternalInput")
    z_d = nc.dram_tensor("z_out", (S, BS, D), FP32, kind="ExternalOutput")
    mu_d = nc.dram_tensor("mu_out", (S, BS, D), FP32, kind="ExternalOutput")
    sc_d = nc.dram_tensor("sc_out", (S, BS, D), FP32, kind="ExternalOutput")

    AF = mybir.ActivationFunctionType
    OP = mybir.AluOpType

    with tile.TileContext(nc) as tc, ExitStack() as ctx:
        singles = ctx.enter_context(tc.tile_pool(name="singles", bufs=1))
        ablk_pool = ctx.enter_context(tc.tile_pool(name="ablk", bufs=3))
        scratch = ctx.enter_context(tc.tile_pool(name="scratch", bufs=3))
        psPre = ctx.enter_context(tc.tile_pool(name="psPre", bufs=2,
                                               space="PSUM"))
        psA = ctx.enter_context(tc.tile_pool(name="psA", bufs=2, space="PSUM"))
        psOut = ctx.enter_context(tc.tile_pool(name="psOut", bufs=1,
                                               space="PSUM"))
        psSc = ctx.enter_context(tc.tile_pool(name="psSc", bufs=1,
                                              space="PSUM"))

        # ---- load inputs/constants into SBUF (spread across the SP,
        # Act and GPSIMD DGE queues so descriptor generation overlaps) ----
        wct_sb = singles.tile([128, 2, HP], BF16)
        for q in range(4):
            nc.sync.dma_start(
                wct_sb[:, :, 512 * q:512 * q + 512],
                wct_d.ap()[:, 512 * q:512 * q + 512].rearrange(
                    "(k p) h -> p k h", p=128))
        ctx_sb = singles.tile([BS, CTX], FP32)
        nc.scalar.dma_start(ctx_sb[:], ctx_d.ap())
        ident_sb = singles.tile([128, 128], F32R)
        nc.scalar.dma_start(ident_sb[:], ident_d.ap())
        b1_sb = singles.tile([128, HP // 128], FP32)
        nc.scalar.dma_start(b1_sb[:],
                            b1_d.ap().rearrange("(c p) one -> p (c one)",
                                                p=128))
        boutb_sb = singles.tile([128, 2 * D], FP32)
        nc.scalar.dma_start(boutb_sb[:], boutb_d.ap())
        w1tb_sb = singles.tile([32, 2, HP], BF16)
        nc.sync.dma_start(w1tb_sb[:], w1tb_d.ap())
        woutt_sb = singles.tile([32, D - 1, 2 * D], BF16)
        nc.sync.dma_start(woutt_sb[:], woutt_d.ap())
        eps2 = [singles.tile([BS, 2, D], FP32, tag=f"eps{h}", name=f"eps{h}")
                for h in (0, 1)]
        for h in (0, 1):
            nc.gpsimd.dma_start(
                eps2[h][:],
                eps_d.ap()[2 * h:2 * h + 2].rearrange("s b d -> b s d"))
        boutbb_sb = singles.tile([1, 2 * D], BF16)
        nc.vector.tensor_copy(boutbb_sb[:], boutb_sb[0:1, :])
        identb_sb = singles.tile([128, 128], BF16)
        nc.vector.tensor_copy(identb_sb[:], ident_sb[:])
        ones_sb = singles.tile([1, 128], BF16)
        nc.vector.memset(ones_sb[:], 1.0)

        # ---- ctx in bf16 (rounded copy) for the transpose ----
        ctxr_sb = singles.tile([BS, CTX], BF16)
        nc.vector.tensor_copy(ctxr_sb[:], ctx_sb[:])

        # ---- ctxT: (BS, CTX) -> (CTX, BS) in 2 chunks ----
        ctxT_sb = singles.tile([128, 2, BS], BF16)
        for k in range(2):
            ps = psPre.tile([128, BS], BF16, tag="apspre")
            nc.tensor.transpose(ps[:], ctxr_sb[:, k * 128:(k + 1) * 128],
                                identb_sb[:])
            nc.vector.tensor_copy(ctxT_sb[:, k, :], ps[:])

        # ---- A_base = WcT.T @ ctxT + b1 : (HP, BS) in 16 unit-chunks ----
        NCH = HP // 128
        a_base = singles.tile([128, NCH, BS], F32R)
        for hc in range(NCH):
            ps = psPre.tile([128, BS], FP32, tag="apspre")
            for k in range(2):
                nc.tensor.matmul(
                    ps[:],
                    wct_sb[:, k, hc * 128:(hc + 1) * 128],
                    ctxT_sb[:, k, :],
                    start=(k == 0), stop=(k == 1))
            nc.vector.tensor_scalar_add(a_base[:, hc, :], ps[:],
                                        b1_sb[:, hc:hc + 1])

        # ---- per-half state ----
        # z2: (b, hwin, s, beta) bf16; zq: (32g+beta, hwin, s, a) bf16
        z2 = [singles.tile([BS, 2, 64], BF16, tag=f"z{h}", name=f"z{h}")
              for h in (0, 1)]
        zq2 = [[singles.tile([32, 4, 2, 32], BF16, tag=f"zq{h}w{w}",
                             name=f"zq{h}w{w}") for w in (0, 1)]
               for h in (0, 1)]
        mu2 = [singles.tile([BS, 2, D], FP32, tag=f"mu{h}", name=f"mu{h}")
               for h in (0, 1)]
        sc2 = [singles.tile([BS, 2, D], FP32, tag=f"sc{h}", name=f"sc{h}")
               for h in (0, 1)]
        zf2 = [singles.tile([BS, 2, D], FP32, tag=f"zf{h}", name=f"zf{h}")
               for h in (0, 1)]
        outr = psOut.tile([128, 4, 128], FP32, tag="outr", name="outr")
        # sc in PSUM: cols 0..63 = scale_i, col 64 = exp scratch
        scps = psSc.tile([128, 4, 65], FP32, tag="scps", name="scps")

        for h in (0, 1):
            nc.vector.memset(z2[h][:], 0.0)
            nc.vector.memset(zq2[h][0][:], 0.0)
            nc.vector.memset(zq2[h][1][:], 0.0)

        def bridge(i, h):
            """StreamTranspose the 32-degree window containing col i from
            z-batch layout into matmul rhs layout: zq[beta, w, g, s, a] =
            z[32g+a, s, 32w+beta] (4 per-batch-group block transposes,
            SBUF->SBUF, all landing at partitions 0..31)."""
            w = i // 32
            for g in (0, 1, 2, 3):
                nc.vector.transpose(zq2[h][w][0:32, g, :, :],
                                    z2[h][32 * g:32 * g + 32, w, :])

        def zupdate(i, h, mu_ap):
            """z_i = mu_i + sc_i*eps_i as Act ops (one per s)."""
            w, beta = i // 32, i % 32
            for s in (0, 1):
                nc.scalar.activation(
                    out=z2[h][:, w, 32 * s + beta:32 * s + beta + 1],
                    in_=scps[:, 2 * h + s, i:i + 1],
                    func=AF.Identity,
                    bias=mu_ap(s),
                    scale=eps2[h][:, s, i:i + 1])

        # ---- step 0 (bias-only): mu0 = bout[0], sc0 = softplus(bout[D]) ----
        for h in (0, 1):
            nc.scalar.activation(out=scps[:, 2 * h:2 * h + 2, 64],
                                 in_=boutb_sb[:, D:D + 1].to_broadcast((BS, 2)),
                                 func=AF.Exp, bias=0.0, scale=1.0)
            nc.scalar.activation(out=scps[:, 2 * h:2 * h + 2, 0],
                                 in_=scps[:, 2 * h:2 * h + 2, 64],
                                 func=AF.Ln, bias=1.0, scale=1.0)
            zupdate(0, h, lambda s: boutb_sb[:, 0:1])
            bridge(0, h)

        # ---- seed OUT with bout once (ones x bout row); single psum
        # accumulation group for the whole shared bank ----
        for hs in range(4):
            nc.tensor.matmul(outr[:, hs, :], ones_sb[:], boutbb_sb[:],
                             start=(hs == 0), stop=False,
                             skip_group_check=True)

        # ---- steps 1..63, two interleaved half-chains ----
        for i in range(1, D):
            nn = int(cnt[i])
            pp = 32 * (i - 1)              # padded unit offset of block i
            c, pl = pp // 128, pp % 128
            q = min(pl, 64)                # 32-aligned base (96 -> 64)
            kk = pl - q + nn
            wfull = i // 32                # complete 32-windows in history
            m = i % 32                     # rows of the partial window
            for h in (0, 1):
                # fresh block pre-activation = ctx_base (shifted-identity MM)
                # + sum over history windows of W1Tb.T @ zq
                aps_t = psA.tile([32, 256], FP32, tag=f"aps{h}")
                sl = a_base[q:q + kk, c, :]
                rhs = bass.AP(sl.tensor, sl.offset,
                              [sl.ap[0], [32, 4], [0, 2], [1, 32]])
                nc.tensor.matmul(aps_t[0:nn, :],
                                 ident_sb[q:q + kk, pl:pl + nn],
                                 rhs, start=True, stop=False,
                                 skip_group_check=True)
                nmm = (1 if wfull else 0) + (1 if m else 0)
                j = 0
                for w in range(wfull):     # complete windows (static
                    # zq, runs early off-chain)
                    j += 1
                    nc.tensor.matmul(
                        aps_t[0:nn, 0:128],
                        w1tb_sb[0:32, w, pp:pp + nn],
                        zq2[h][w][0:32, 0:2, :, :],
                        start=False, stop=False,
                        skip_group_check=True)
                    nc.tensor.matmul(
                        aps_t[0:nn, 128:256],
                        w1tb_sb[0:32, w, pp:pp + nn],
                        zq2[h][w][0:32, 2:4, :, :],
                        start=False, stop=(j == nmm),
                        skip_group_check=True)
                if m:                      # partial window, 2 g-pair
                    # chunks so the first overlaps the later bridges
                    j += 1
                    nc.tensor.matmul(
                        aps_t[0:nn, 0:128],
                        w1tb_sb[0:m, wfull, pp:pp + nn],
                        zq2[h][wfull][0:m, 0:2, :, :],
                        start=False, stop=False,
                        skip_group_check=True)
                    nc.tensor.matmul(
                        aps_t[0:nn, 128:256],
                        w1tb_sb[0:m, wfull, pp:pp + nn],
                        zq2[h][wfull][0:m, 2:4, :, :],
                        start=False, stop=(j == nmm),
                        skip_group_check=True)

                # relu -> bf16 on Act, reordering (g,s,a) -> (s,g,a) so
                # the contrib lhsT per s is a contiguous 128-col slice
                ab = ablk_pool.tile([32, 256], BF16, tag=f"ablk{h}")
                asl = aps_t[0:nn, 0:1]
                ain0 = bass.AP(asl.tensor, asl.offset,
                               [asl.ap[0], [64, 4], [1, 32]])
                ain1 = bass.AP(asl.tensor, asl.offset + 32,
                               [asl.ap[0], [64, 4], [1, 32]])
                nc.scalar.activation(out=ab[0:nn, 128:256], in_=ain1,
                                     func=AF.Relu, bias=0.0, scale=1.0)
                nc.vector.tensor_scalar_max(ab[0:nn, 0:128], ain0, 0.0)

                # per-s tail, entirely on Act in (128,1) columns (zero
                # marginal engine time): narrow contrib -> mu copy to SBUF
                # -> softplus -> z = Identity(sc*eps + mu)
                wsl = woutt_sb[0:nn, i - 1, :]
                w_, beta = i // 32, i % 32
                musb = scratch.tile([BS, 2], FP32, tag=f"mu{h}s")
                for s in (1, 0):
                    lhs_sl = ab[0:nn, 128 * s:128 * s + 128]
                    wap_n = bass.AP(wsl.tensor, wsl.offset + i,
                                    [wsl.ap[0], [D, 2], [1, 1]])
                    osl = outr[:, 2 * h + s, :]
                    oap_n = bass.AP(osl.tensor, osl.offset + i,
                                    [osl.ap[0], [D, 2], [1, 1]])
                    nc.tensor.matmul(oap_n, lhs_sl, wap_n,
                                     start=False,
                                     stop=(i == D - 1 and s == 0 and h == 1),
                                     skip_group_check=True)
                    nc.scalar.activation(out=musb[:, s:s + 1],
                                         in_=outr[:, 2 * h + s, i:i + 1],
                                         func=AF.Copy, bias=0.0, scale=1.0)
                    nc.scalar.activation(out=scps[:, 2 * h + s, 64:65],
                                         in_=outr[:, 2 * h + s,
                                                  D + i:D + i + 1],
                                         func=AF.Exp, bias=0.0, scale=1.0)
                    nc.scalar.activation(out=scps[:, 2 * h + s, i:i + 1],
                                         in_=scps[:, 2 * h + s, 64:65],
                                         func=AF.Ln, bias=1.0, scale=1.0)
                    nc.scalar.activation(
                        out=z2[h][:, w_, 32 * s + beta:32 * s + beta + 1],
                        in_=scps[:, 2 * h + s, i:i + 1],
                        func=AF.Identity,
                        bias=musb[:, s:s + 1],
                        scale=eps2[h][:, s, i:i + 1])
                if i < D - 1:
                    bridge(i, h)

                # deferred wide contribution: cols {i+1..D-1, D+i+1..2D-1}
                if i < D - 1:
                    for s in (0, 1):
                        lhs_sl = ab[0:nn, 128 * s:128 * s + 128]
                        wap_w = bass.AP(wsl.tensor, wsl.offset + i + 1,
                                        [wsl.ap[0], [D, 2], [1, D - 1 - i]])
                        osl = outr[:, 2 * h + s, :]
                        oap_w = bass.AP(osl.tensor, osl.offset + i + 1,
                                        [osl.ap[0], [D, 2], [1, D - 1 - i]])
                        nc.tensor.matmul(oap_w, lhs_sl, wap_w,
                                         start=False, stop=False,
                                         skip_group_check=True)

        # ---- mu/sc extraction (batched) + outputs ----
        for h in (0, 1):
            nc.vector.tensor_copy(mu2[h][:, :, 0],
                                  boutb_sb[:, 0:1].to_broadcast((BS, 2)))
            nc.vector.tensor_copy(mu2[h][:, :, 1:D], outr[:, 2 * h:2 * h + 2, 1:D])
            nc.vector.tensor_copy(sc2[h][:], scps[:, 2 * h:2 * h + 2, 0:D])
            # zf[b, s, d] = z2[b, d//32, 32*s + d%32] (bf16->f32 gather)
            zsl = z2[h][:, 0, 0:1]
            zin = bass.AP(zsl.tensor, zsl.offset,
                          [zsl.ap[0], [32, 2], [64, 2], [1, 32]])
            fsl = zf2[h][:, 0, 0:1]
            fout = bass.AP(fsl.tensor, fsl.offset,
                           [fsl.ap[0], [64, 2], [32, 2], [1, 32]])
            nc.vector.tensor_copy(fout, zin)
            nc.sync.dma_start(
                z_d.ap()[2 * h:2 * h + 2].rearrange("s b d -> b s d"),
                zf2[h][:])
            nc.gpsimd.dma_start(
                mu_d.ap()[2 * h:2 * h + 2].rearrange("s b d -> b s d"),
                mu2[h][:])
            nc.scalar.dma_start(
                sc_d.ap()[2 * h:2 * h + 2].rearrange("s b d -> b s d"),
                sc2[h][:])

    nc.compile()
    _PROGRAM_CACHE = nc
    return nc


def _in_maps(context, eps, W1, b1, Wc, Wout, bout):
    W1Tb, WoutB, WcT, b1p = _prep_weights(W1, b1, Wc, Wout)
    ident = np.eye(128, dtype=np.float32)
    boutb = np.ascontiguousarray(np.tile(bout.reshape(1, -1), (128, 1)))
    maps = []
    for c in range(NCORES):
        maps.append({
            "ctx": np.ascontiguousarray(context[c * BS:(c + 1) * BS]),
            "eps": np.ascontiguousarray(eps[:, c * BS:(c + 1) * BS]),
            "w1tb": W1Tb, "woutt": WoutB, "wct": WcT, "b1": b1p,
            "boutb": boutb, "ident": ident,
        })
    return maps


def run(context, eps, W1, b1, Wc, Wout, bout, trace=False):
    context = np.asarray(context, np.float32)
    eps = np.asarray(eps, np.float32)
    W1 = np.asarray(W1, np.float32)
    b1 = np.asarray(b1, np.float32)
    Wc = np.asarray(Wc, np.float32)
    Wout = np.asarray(Wout, np.float32)
    bout = np.asarray(bout, np.float32)
    nc = _build_program()
    maps = _in_maps(context, eps, W1, b1, Wc, Wout, bout)
    res = run_bass_kernel_spmd(nc, maps, core_ids=list(range(NCORES)),
                               trace=trace)
    z = np.empty((S, B, D), np.float32)
    mu = np.empty((S, B, D), np.float32)
    sc = np.empty((S, B, D), np.float32)
    for c in range(NCORES):
        z[:, c * BS:(c + 1) * BS] = res.results[c]["z_out"]
        mu[:, c * BS:(c + 1) * BS] = res.results[c]["mu_out"]
        sc[:, c * BS:(c + 1) * BS] = res.results[c]["sc_out"]
    return (z, mu, sc), res


def kernel(context, eps, W1, b1, Wc, Wout, bout):
    (z, mu, sc), _ = run(context, eps, W1, b1, Wc, Wout, bout)
    return z, mu, sc


# revision 4
# speedup vs baseline: 1.4111x; 1.0003x over previous
"""Trainium2 Bass kernel for nn_AutoRegressiveDistribution (MADE sampling).

Self-contained: hardcodes shapes/sharding. Shards batch B across 8 cores,
runs the D-step autoregressive sampling loop fully on-device per core.

Per-core structure (rows = S*BS = 512, processed as TWO independent
half-chains of an s-pair each so the serial per-step dependency chains
overlap across engines). v2 critical-path redesign:
  - z history is kept bf16 in a 32-blocked layout z2 (b, h, s, beta) and
    bridged to the matmul-consumable layout zq (32g+beta, h, s, a) by ONE
    DVE StreamTranspose of the current 32-degree window per step (SBUF->
    SBUF, ~130ns) instead of PE transposes + a PSUM->SBUF copy (~760ns).
    The hist matmul contracts over (g,beta) partitions per batch-group g:
    permuting the contraction dim identically in lhsT (W1Tb) and rhs (zq)
    leaves the sum invariant, so 4-8 small bf16 MMs replace one big one
    (matmul cost is out-free-size bound, so total PE time is unchanged).
  - The z-update z_i = mu_i + sc_i*eps_i runs on the GPSIMD/Pool engine as
    two fused scalar_tensor_tensor ops (one per s): Pool has no SBUF/PSUM
    access-latency penalty, so reading mu from PSUM is cheap there.
  - softplus = Exp then Ln(1+x) on the Act engine, entirely in PSUM
    (scratch col 64 of the sc tile), avoiding the 222-cycle SBUF access.
  - Only the 2 output columns needed for the z-update ({i, D+i}) are
    accumulated on the critical path (narrow contrib MM); the remaining
    columns (> i) are added by a deferred wide MM off the critical path.
  - ctx_h = Wc @ ctx + b1 is precomputed once (f32r identity-shift seed
    MMs); bout is seeded into OUT once via ones-outer-product matmuls.
"""

import numpy as np
from contextlib import ExitStack

import concourse.bass as bass
import concourse.tile as tile
from concourse import bacc, mybir
from concourse.bass_utils import run_bass_kernel_spmd

D, H, CTX, B, S = 64, 1024, 256, 1024, 4
NCORES = 8
BS = B // NCORES          # 128 batch rows per core
R = S * BS                # 512 rows per core
RH = R // 2               # rows per half-chain (s-pair)
FP32 = mybir.dt.float32
BF16 = mybir.dt.bfloat16
F32R = mybir.dt.float32r

HP = 2048  # padded hidden units: degree block i at [32*(i-1), 32*(i-1)+cnt[i])


def _made_struct():
    mh = (np.arange(H) % (D - 1)) + 1            # degrees 1..63
    perm = np.argsort(mh, kind="stable")
    mh_s = mh[perm]
    cnt = np.bincount(mh_s, minlength=D)          # cnt[d] = #units of degree d
    off = np.concatenate([[0], np.cumsum(cnt)[:-1]]).astype(np.int64)
    return mh, perm, mh_s, cnt, off


def _prep_weights(W1, b1, Wc, Wout):
    """Mask + permute + 32-pad weights host-side (cheap, O(weight size))."""
    mh, perm, mh_s, cnt, off = _made_struct()
    m0 = np.arange(1, D + 1)
    M1 = (mh[:, None] >= m0[None, :]).astype(np.float32)          # (H, D)
    mout = np.concatenate([m0, m0])                                # (2D,)
    Mout = (mout[:, None] > mh[None, :]).astype(np.float32)        # (2D, H)
    W1m = (W1 * M1)[perm]                   # (H, D) permuted rows
    Woutm = (Wout * Mout)[:, perm]          # (2D, H) permuted cols
    src = np.arange(H)
    pdst = 32 * (mh_s - 1) + (src - off[mh_s])   # padded slot of sorted unit
    import ml_dtypes
    bf = ml_dtypes.bfloat16
    W1T = np.zeros((D, HP), np.float32)
    W1T[:, pdst] = W1m.T
    # blocked lhsT for the hist MMs: W1Tb[beta, h, u] = W1T[32h+beta, u]
    W1Tb = np.ascontiguousarray(
        W1T.reshape(2, 32, HP).transpose(1, 0, 2)).astype(bf)
    WcT = np.zeros((CTX, HP), np.float32)
    WcT[:, pdst] = Wc[perm].T
    WcT = WcT.astype(bf)
    b1p = np.zeros((HP, 1), np.float32)
    b1p[pdst, 0] = b1[perm]
    WoutB = np.zeros((32, D - 1, 2 * D), np.float32)  # (slot, block, outcol)
    WoutB[pdst % 32, (mh_s - 1)] = Woutm[:, :].T[src]
    return W1Tb, WoutB.astype(bf), WcT, b1p


_PROGRAM_CACHE = None


def _pin_act_table():
    """Make Exp/Ln/Relu resolvable only via natural_log_exp_and_others so
    the act-table chooser doesn't thrash between the exp and ln tables
    (each LoadActFuncSet costs ~1.3us). Table positions are preserved so
    act_func_set_id stays consistent with act_info.json."""
    import concourse.bacc as bacc_mod
    from concourse import hw_specs
    orig = hw_specs.get_activation_tables
    AF = mybir.ActivationFunctionType
    pin = {AF.Exp, AF.Ln, AF.Relu, AF.Copy, AF.Identity}

    def filtered(arch):
        out = {}
        for name, fns in orig(arch).items():
            if name == "natural_log_exp_and_others":
                out[name] = set(fns)
            else:
                out[name] = set(fns) - pin
        return out

    bacc_mod.get_activation_tables = filtered


def _build_program():
    """Build + compile the SPMD Bass program (input-independent, cached)."""
    global _PROGRAM_CACHE
    if _PROGRAM_CACHE is not None:
        return _PROGRAM_CACHE
    _pin_act_table()
    _, _, mh_s, cnt, off = _made_struct()

    nc = bacc.Bacc("TRN2", target_bir_lowering=False, debug=False,
                   num_devices=NCORES)

    ctx_d = nc.dram_tensor("ctx", (BS, CTX), FP32, kind="ExternalInput")
    eps_d = nc.dram_tensor("eps", (S, BS, D), FP32, kind="ExternalInput")
    w1tb_d = nc.dram_tensor("w1tb", (32, 2, HP), BF16, kind="ExternalInput")
    woutt_d = nc.dram_tensor("woutt", (32, D - 1, 2 * D), BF16,
                             kind="ExternalInput")
    wct_d = nc.dram_tensor("wct", (CTX, HP), BF16, kind="ExternalInput")
    b1_d = nc.dram_tensor("b1", (HP, 1), FP32, kind="ExternalInput")
    boutb_d = nc.dram_tensor("boutb", (128, 2 * D), FP32, kind="ExternalInput")
    ident_d = nc.dram_tensor("ident", (128, 128), F32R, kind="ExternalInput")
    z_d = nc.dram_tensor("z_out", (S, BS, D), FP32, kind="ExternalOutput")
    mu_d = nc.dram_tensor("mu_out", (S, BS, D), FP32, kind="ExternalOutput")
    sc_d = nc.dram_tensor("sc_out", (S, BS, D), FP32, kind="ExternalOutput")

    AF = mybir.ActivationFunctionType
    OP = mybir.AluOpType

    with tile.TileContext(nc) as tc, ExitStack() as ctx:
        singles = ctx.enter_context(tc.tile_pool(name="singles", bufs=1))
        ablk_pool = ctx.enter_context(tc.tile_pool(name="ablk", bufs=3))
        scratch = ctx.enter_context(tc.tile_pool(name="scratch", bufs=3))
        psPre = ctx.enter_context(tc.tile_pool(name="psPre", bufs=2,
                                               space="PSUM"))
        psA = ctx.enter_context(tc.tile_pool(name="psA", bufs=2, space="PSUM"))
        psOut = ctx.enter_context(tc.tile_pool(name="psOut", bufs=1,
                                               space="PSUM"))
        psSc = ctx.enter_context(tc.tile_pool(name="psSc", bufs=1,
                                              space="PSUM"))

        # ---- load inputs/constants into SBUF (spread across the SP,
        # Act and GPSIMD DGE queues so descriptor generation overlaps) ----
        wct_sb = singles.tile([128, 2, HP], BF16)
        nc.sync.dma_start(
            wct_sb[:, :, 0:512],
            wct_d.ap()[:, 0:512].rearrange("(k p) h -> p k h", p=128))
        boutb_sb = singles.tile([128, 2 * D], FP32)
        nc.scalar.dma_start(boutb_sb[:], boutb_d.ap())
        ctx_sb = singles.tile([BS, CTX], FP32)
        nc.sync.dma_start(ctx_sb[:], ctx_d.ap())
        ident_sb = singles.tile([128, 128], F32R)
        nc.sync.dma_start(ident_sb[:], ident_d.ap())
        for q in range(1, 4):
            nc.sync.dma_start(
                wct_sb[:, :, 512 * q:512 * q + 512],
                wct_d.ap()[:, 512 * q:512 * q + 512].rearrange(
                    "(k p) h -> p k h", p=128))
        w1tb_sb = singles.tile([32, 2, HP], BF16)
        nc.sync.dma_start(w1tb_sb[:], w1tb_d.ap())
        woutt_sb = singles.tile([32, D - 1, 2 * D], BF16)
        nc.sync.dma_start(woutt_sb[:], woutt_d.ap())
        eps2 = [singles.tile([BS, 2, D], FP32, tag=f"eps{h}", name=f"eps{h}")
                for h in (0, 1)]
        nc.gpsimd.dma_start(
            eps2[0][:], eps_d.ap()[0:2].rearrange("s b d -> b s d"))
        nc.scalar.dma_start(
            eps2[1][:], eps_d.ap()[2:4].rearrange("s b d -> b s d"))
        b1_sb = singles.tile([128, HP // 128], FP32)
        nc.gpsimd.dma_start(b1_sb[:],
                            b1_d.ap().rearrange("(c p) one -> p (c one)",
                                                p=128))
        boutbb_sb = singles.tile([1, 2 * D], BF16)
        nc.vector.tensor_copy(boutbb_sb[:], boutb_sb[0:1, :])
        identb_sb = singles.tile([128, 128], BF16)
        nc.vector.tensor_copy(identb_sb[:], ident_sb[:])
        ones_sb = singles.tile([1, 128], BF16)
        nc.vector.memset(ones_sb[:], 1.0)

        # ---- per-half state ----
        # z2: (b, hwin, s, beta) bf16; zq: (32g+beta, hwin, s, a) bf16
        z2 = [singles.tile([BS, 2, 64], BF16, tag=f"z{h}", name=f"z{h}")
              for h in (0, 1)]
        zq2 = [[singles.tile([32, 4, 2, 32], BF16, tag=f"zq{h}w{w}",
                             name=f"zq{h}w{w}") for w in (0, 1)]
               for h in (0, 1)]
        mu2 = [singles.tile([BS, 2, D], FP32, tag=f"mu{h}", name=f"mu{h}")
               for h in (0, 1)]
        sc2 = [singles.tile([BS, 2, D], FP32, tag=f"sc{h}", name=f"sc{h}")
               for h in (0, 1)]
        zf2 = [singles.tile([BS, 2, D], FP32, tag=f"zf{h}", name=f"zf{h}")
               for h in (0, 1)]
        outr = psOut.tile([128, 4, 128], FP32, tag="outr", name="outr")
        # sc in PSUM: cols 0..63 = scale_i, col 64 = exp scratch
        scps = psSc.tile([128, 4, 65], FP32, tag="scps", name="scps")

        for h in (0, 1):
            nc.vector.memset(z2[h][:], 0.0)
            nc.vector.memset(zq2[h][0][:], 0.0)
            nc.vector.memset(zq2[h][1][:], 0.0)

        # ---- ctx in bf16 (rounded copy) for the transpose ----
        ctxr_sb = singles.tile([BS, CTX], BF16)
        nc.vector.tensor_copy(ctxr_sb[:], ctx_sb[:])

        # ---- ctxT: (BS, CTX) -> (CTX, BS) in 2 chunks ----
        ctxT_sb = singles.tile([128, 2, BS], BF16)
        for k in range(2):
            ps = psPre.tile([128, BS], BF16, tag="apspre")
            nc.tensor.transpose(ps[:], ctxr_sb[:, k * 128:(k + 1) * 128],
                                identb_sb[:])
            nc.vector.tensor_copy(ctxT_sb[:, k, :], ps[:])

        # ---- A_base = WcT.T @ ctxT + b1 : (HP, BS) in 16 unit-chunks ----
        NCH = HP // 128
        a_base = singles.tile([128, NCH, BS], F32R)
        for hc in range(NCH):
            ps = psPre.tile([128, BS], FP32, tag="apspre")
            for k in range(2):
                nc.tensor.matmul(
                    ps[:],
                    wct_sb[:, k, hc * 128:(hc + 1) * 128],
                    ctxT_sb[:, k, :],
                    start=(k == 0), stop=(k == 1))
            nc.vector.tensor_scalar_add(a_base[:, hc, :], ps[:],
                                        b1_sb[:, hc:hc + 1])

        def bridge(i, h):
            """StreamTranspose the 32-degree window containing col i from
            z-batch layout into matmul rhs layout: zq[beta, w, g, s, a] =
            z[32g+a, s, 32w+beta] (4 per-batch-group block transposes,
            SBUF->SBUF, all landing at partitions 0..31)."""
            w = i // 32
            for g in (0, 1, 2, 3):
                nc.vector.transpose(zq2[h][w][0:32, g, :, :],
                                    z2[h][32 * g:32 * g + 32, w, :])

        def zupdate(i, h, mu_ap):
            """z_i = mu_i + sc_i*eps_i as Act ops (one per s)."""
            w, beta = i // 32, i % 32
            for s in (0, 1):
                nc.scalar.activation(
                    out=z2[h][:, w, 32 * s + beta:32 * s + beta + 1],
                    in_=scps[:, 2 * h + s, i:i + 1],
                    func=AF.Identity,
                    bias=mu_ap(s),
                    scale=eps2[h][:, s, i:i + 1])

        # ---- step 0 (bias-only): mu0 = bout[0], sc0 = softplus(bout[D]) ----
        for h in (0, 1):
            nc.scalar.activation(out=scps[:, 2 * h:2 * h + 2, 64],
                                 in_=boutb_sb[:, D:D + 1].to_broadcast((BS, 2)),
                                 func=AF.Exp, bias=0.0, scale=1.0)
            nc.scalar.activation(out=scps[:, 2 * h:2 * h + 2, 0],
                                 in_=scps[:, 2 * h:2 * h + 2, 64],
                                 func=AF.Ln, bias=1.0, scale=1.0)
            zupdate(0, h, lambda s: boutb_sb[:, 0:1])
            bridge(0, h)

        # ---- seed OUT with bout once (ones x bout row); single psum
        # accumulation group for the whole shared bank ----
        for hs in range(4):
            nc.tensor.matmul(outr[:, hs, :], ones_sb[:], boutbb_sb[:],
                             start=(hs == 0), stop=False,
                             skip_group_check=True)

        # ---- steps 1..63, two interleaved half-chains ----
        for i in range(1, D):
            nn = int(cnt[i])
            pp = 32 * (i - 1)              # padded unit offset of block i
            c, pl = pp // 128, pp % 128
            q = min(pl, 64)                # 32-aligned base (96 -> 64)
            kk = pl - q + nn
            wfull = i // 32                # complete 32-windows in history
            m = i % 32                     # rows of the partial window
            for h in (0, 1):
                # fresh block pre-activation = ctx_base (shifted-identity MM)
                # + sum over history windows of W1Tb.T @ zq
                aps_t = psA.tile([32, 256], FP32, tag=f"aps{h}")
                sl = a_base[q:q + kk, c, :]
                rhs = bass.AP(sl.tensor, sl.offset,
                              [sl.ap[0], [32, 4], [0, 2], [1, 32]])
                nc.tensor.matmul(aps_t[0:nn, :],
                                 ident_sb[q:q + kk, pl:pl + nn],
                                 rhs, start=True, stop=False,
                                 skip_group_check=True)
                nmm = (1 if wfull else 0) + (1 if m else 0)
                j = 0
                for w in range(wfull):     # complete windows (static
                    # zq, runs early off-chain)
                    j += 1
                    nc.tensor.matmul(
                        aps_t[0:nn, 0:128],
                        w1tb_sb[0:32, w, pp:pp + nn],
                        zq2[h][w][0:32, 0:2, :, :],
                        start=False, stop=False,
                        skip_group_check=True)
                    nc.tensor.matmul(
                        aps_t[0:nn, 128:256],
                        w1tb_sb[0:32, w, pp:pp + nn],
                        zq2[h][w][0:32, 2:4, :, :],
                        start=False, stop=(j == nmm),
                        skip_group_check=True)
                if m:                      # partial window, 2 g-pair
                    # chunks so the first overlaps the later bridges
                    j += 1
                    nc.tensor.matmul(
                        aps_t[0:nn, 0:128],
                        w1tb_sb[0:m, wfull, pp:pp + nn],
                        zq2[h][wfull][0:m, 0:2, :, :],
                        start=False, stop=False,
                        skip_group_check=True)
                    nc.tensor.matmul(
                        aps_t[0:nn, 128:256],
                        w1tb_sb[0:m, wfull, pp:pp + nn],
                        zq2[h][wfull][0:m, 2:4, :, :],
                        start=False, stop=(j == nmm),
                        skip_group_check=True)

                # relu -> bf16 on Act, reordering (g,s,a) -> (s,g,a) so
                # the contrib lhsT per s is a contiguous 128-col slice
                ab = ablk_pool.tile([32, 256], BF16, tag=f"ablk{h}")
                asl = aps_t[0:nn, 0:1]
                ain0 = bass.AP(asl.tensor, asl.offset,
                               [asl.ap[0], [64, 4], [1, 32]])
                ain1 = bass.AP(asl.tensor, asl.offset + 32,
                               [asl.ap[0], [64, 4], [1, 32]])
                nc.scalar.activation(out=ab[0:nn, 128:256], in_=ain1,
                                     func=AF.Relu, bias=0.0, scale=1.0)
                nc.vector.tensor_scalar_max(ab[0:nn, 0:128], ain0, 0.0)

                # per-s tail, entirely on Act in (128,1) columns (zero
                # marginal engine time): narrow contrib -> mu copy to SBUF
                # -> softplus -> z = Identity(sc*eps + mu)
                wsl = woutt_sb[0:nn, i - 1, :]
                w_, beta = i // 32, i % 32
                musb = scratch.tile([BS, 2], FP32, tag=f"mu{h}s")
                for s in (1, 0):
                    lhs_sl = ab[0:nn, 128 * s:128 * s + 128]
                    wap_n = bass.AP(wsl.tensor, wsl.offset + i,
                                    [wsl.ap[0], [D, 2], [1, 1]])
                    osl = outr[:, 2 * h + s, :]
                    oap_n = bass.AP(osl.tensor, osl.offset + i,
                                    [osl.ap[0], [D, 2], [1, 1]])
                    nc.tensor.matmul(oap_n, lhs_sl, wap_n,
                                     start=False,
                                     stop=(i == D - 1 and s == 0 and h == 1),
                                     skip_group_check=True)
                    nc.scalar.activation(out=musb[:, s:s + 1],
                                         in_=outr[:, 2 * h + s, i:i + 1],
                                         func=AF.Copy, bias=0.0, scale=1.0)
                    nc.scalar.activation(out=scps[:, 2 * h + s, 64:65],
                                         in_=outr[:, 2 * h + s,
                                                  D + i:D + i + 1],
                                         func=AF.Exp, bias=0.0, scale=1.0)
                    nc.scalar.activation(out=scps[:, 2 * h + s, i:i + 1],
                                         in_=scps[:, 2 * h + s, 64:65],
                                         func=AF.Ln, bias=1.0, scale=1.0)
                    nc.scalar.activation(
                        out=z2[h][:, w_, 32 * s + beta:32 * s + beta + 1],
                        in_=scps[:, 2 * h + s, i:i + 1],
                        func=AF.Identity,
                        bias=musb[:, s:s + 1],
                        scale=eps2[h][:, s, i:i + 1])
                if i < D - 1:
                    bridge(i, h)

                # deferred wide contribution: cols {i+1..D-1, D+i+1..2D-1}
                if i < D - 1:
                    for s in (0, 1):
                        lhs_sl = ab[0:nn, 128 * s:128 * s + 128]
                        wap_w = bass.AP(wsl.tensor, wsl.offset + i + 1,
                                        [wsl.ap[0], [D, 2], [1, D - 1 - i]])
                        osl = outr[:, 2 * h + s, :]
                        oap_w = bass.AP(osl.tensor, osl.offset + i + 1,
                                        [osl.ap[0], [D, 2], [1, D - 1 - i]])
                        nc.tensor.matmul(oap_w, lhs_sl, wap_w,
                                         start=False, stop=False,
                                         skip_group_check=True)

        # ---- mu/sc extraction (batched) + outputs ----
        for h in (0, 1):
            nc.vector.tensor_copy(mu2[h][:, :, 0],
                                  boutb_sb[:, 0:1].to_broadcast((BS, 2)))
            nc.vector.tensor_copy(mu2[h][:, :, 1:D], outr[:, 2 * h:2 * h + 2, 1:D])
            nc.vector.tensor_copy(sc2[h][:], scps[:, 2 * h:2 * h + 2, 0:D])
            # zf[b, s, d] = z2[b, d//32, 32*s + d%32] (bf16->f32 gather)
            zsl = z2[h][:, 0, 0:1]
            zin = bass.AP(zsl.tensor, zsl.offset,
                          [zsl.ap[0], [32, 2], [64, 2], [1, 32]])
            fsl = zf2[h][:, 0, 0:1]
            fout = bass.AP(fsl.tensor, fsl.offset,
                           [fsl.ap[0], [64, 2], [32, 2], [1, 32]])
            nc.vector.tensor_copy(fout, zin)
            nc.sync.dma_start(
                z_d.ap()[2 * h:2 * h + 2].rearrange("s b d -> b s d"),
                zf2[h][:])
            nc.gpsimd.dma_start(
                mu_d.ap()[2 * h:2 * h + 2].rearrange("s b d -> b s d"),
                mu2[h][:])
            nc.scalar.dma_start(
                sc_d.ap()[2 * h:2 * h + 2].rearrange("s b d -> b s d"),
                sc2[h][:])

    nc.compile()
    _PROGRAM_CACHE = nc
    return nc


def _in_maps(context, eps, W1, b1, Wc, Wout, bout):
    W1Tb, WoutB, WcT, b1p = _prep_weights(W1, b1, Wc, Wout)
    ident = np.eye(128, dtype=np.float32)
    boutb = np.ascontiguousarray(np.tile(bout.reshape(1, -1), (128, 1)))
    maps = []
    for c in range(NCORES):
        maps.append({
            "ctx": np.ascontiguousarray(context[c * BS:(c + 1) * BS]),
            "eps": np.ascontiguousarray(eps[:, c * BS:(c + 1) * BS]),
            "w1tb": W1Tb, "woutt": WoutB, "wct": WcT, "b1": b1p,
            "boutb": boutb, "ident": ident,
        })
    return maps


def run(context, eps, W1, b1, Wc, Wout, bout, trace=False):
    context = np.asarray(context, np.float32)
    eps = np.asarray(eps, np.float32)
    W1 = np.asarray(W1, np.float32)
    b1 = np.asarray(b1, np.float32)
    Wc = np.asarray(Wc, np.float32)
    Wout = np.asarray(Wout, np.float32)
    bout = np.asarray(bout, np.float32)
    nc = _build_program()
    maps = _in_maps(context, eps, W1, b1, Wc, Wout, bout)
    res = run_bass_kernel_spmd(nc, maps, core_ids=list(range(NCORES)),
                               trace=trace)
    z = np.empty((S, B, D), np.float32)
    mu = np.empty((S, B, D), np.float32)
    sc = np.empty((S, B, D), np.float32)
    for c in range(NCORES):
        z[:, c * BS:(c + 1) * BS] = res.results[c]["z_out"]
        mu[:, c * BS:(c + 1) * BS] = res.results[c]["mu_out"]
        sc[:, c * BS:(c + 1) * BS] = res.results[c]["sc_out"]
    return (z, mu, sc), res


def kernel(context, eps, W1, b1, Wc, Wout, bout):
    (z, mu, sc), _ = run(context, eps, W1, b1, Wc, Wout, bout)
    return z, mu, sc
